# revision 1
# baseline (speedup 1.0000x reference)
"""Chunked-causal attention with sinks on 8 TRN2 NeuronCores.

Sharding: the 64 (batch, head) pairs are split 8-per-core (data parallel on
B, tensor parallel on H). Each core runs the same Bass program over its 8
pairs x 4 chunks of 1024 tokens.

Per (pair, chunk) the kernel computes, entirely on-chip:
  S_T[k, q] = K @ Q^T          (TensorE, bf16, scores transposed so that the
                                PV matmul can consume exp(S_T) directly)
  P_T       = exp(S_T / sqrt(D))  (ScalarE; no max-subtraction -- softmax is
                                shift-invariant and scores here are O(5), so
                                fp32 exp is exact enough; masked entries are
                                zeroed after the exp)
  O[q, :]   = P_T^T @ [V | 1]  (TensorE; the ones column yields the softmax
                                denominator in column D of the same matmul)
  out       = O[:, :D] / (O[:, D] + exp(sink))
"""

import numpy as np

import concourse.bacc as bacc
import concourse.bass as bass  # noqa: F401
import concourse.mybir as mybir
import concourse.tile as tile
from concourse.bass_utils import run_bass_kernel_spmd
from concourse.masks import make_identity

N_CORES = 8
B, S, H, D = 4, 4096, 16, 128
C = 1024                # chunk size
NCH = S // C            # chunks per sequence
PAIRS = B * H           # 64 (batch, head) pairs
PPC = PAIRS // N_CORES  # pairs per core
P = 128                 # SBUF partitions
T = C // P              # 128-row tiles per chunk
SCALE = 1.0 / float(np.sqrt(D))

F32 = mybir.dt.float32
BF16 = mybir.dt.bfloat16


def _build_program():
    nc = bacc.Bacc("TRN2", target_bir_lowering=False, debug=False)
    q_d = nc.dram_tensor("q", [PPC, S, D], F32, kind="ExternalInput")
    k_d = nc.dram_tensor("k", [PPC, S, D], F32, kind="ExternalInput")
    v_d = nc.dram_tensor("v", [PPC, S, D], F32, kind="ExternalInput")
    es_d = nc.dram_tensor("esink", [PPC, P], F32, kind="ExternalInput")
    out_d = nc.dram_tensor("out", [PPC, S, D], F32, kind="ExternalOutput")

    with tile.TileContext(nc) as tc:
        with (
            tc.tile_pool(name="consts", bufs=1) as consts,
            tc.tile_pool(name="loads", bufs=2) as loads,
            tc.tile_pool(name="bcast", bufs=2) as bpool,
            tc.tile_pool(name="qkt", bufs=2) as qktp,
            tc.tile_pool(name="ptile", bufs=2) as ppool,
            tc.tile_pool(name="outs", bufs=2) as opool,
            tc.tile_pool(name="small", bufs=4) as small,
            tc.tile_pool(name="spsum", bufs=2, space="PSUM") as spsum,
            tc.tile_pool(name="opsum", bufs=2, space="PSUM") as opsum,
            tc.tile_pool(name="tpsum", bufs=2, space="PSUM") as tpsum,
        ):
            ident = consts.tile([P, P], BF16)
            make_identity(nc, ident)

            for pair in range(PPC):
                es_t = small.tile([P, 1], F32, tag="esink")
                nc.sync.dma_start(
                    out=es_t, in_=es_d[pair, :].rearrange("(p o) -> p o", o=1)
                )
                for ch in range(NCH):
                    s0 = ch * C

                    qf = loads.tile([P, T, D], F32, tag="qf")
                    nc.sync.dma_start(
                        out=qf,
                        in_=q_d[pair, s0:s0 + C, :].rearrange("(t p) d -> p t d", p=P),
                    )
                    kf = loads.tile([P, T, D], F32, tag="kf")
                    nc.sync.dma_start(
                        out=kf,
                        in_=k_d[pair, s0:s0 + C, :].rearrange("(t p) d -> p t d", p=P),
                    )
                    vf = loads.tile([P, T, D], F32, tag="vf")
                    nc.sync.dma_start(
                        out=vf,
                        in_=v_d[pair, s0:s0 + C, :].rearrange("(t p) d -> p t d", p=P),
                    )

                    # bf16 casts on GpSimd (keeps DVE/ACT free)
                    qb = bpool.tile([P, T, D], BF16, tag="qb")
                    nc.gpsimd.tensor_copy(qb, qf)
                    kb = bpool.tile([P, T, D], BF16, tag="kb")
                    nc.gpsimd.tensor_copy(kb, kf)
                    vb = bpool.tile([P, T, D + 1], BF16, tag="vb")
                    nc.gpsimd.tensor_copy(vb[:, :, 0:D], vf)
                    nc.gpsimd.memset(vb[:, :, D:D + 1], 1.0)

                    # Q^T / K^T via TensorE transpose (bf16, 1 cyc/row)
                    qt_s = qktp.tile([P, C], BF16, tag="qt")
                    kt_s = qktp.tile([P, C], BF16, tag="kt")
                    for t in range(T):
                        tp = tpsum.tile([P, P], BF16, tag="tp")
                        nc.tensor.transpose(tp, qb[:, t, :], ident)
                        nc.vector.tensor_copy(qt_s[:, t * P:(t + 1) * P], tp)
                        tp2 = tpsum.tile([P, P], BF16, tag="tp")
                        nc.tensor.transpose(tp2, kb[:, t, :], ident)
                        nc.vector.tensor_copy(kt_s[:, t * P:(t + 1) * P], tp2)

                    # Phase 1: scores + exp per key-tile
                    pt_all = ppool.tile([P, T, C], BF16, tag="pt")
                    for kt in range(T):
                        c0 = kt * P
                        st = spsum.tile([P, C], F32, tag="st")
                        # split matmuls at the PSUM bank boundary (col 512)
                        if c0 < 512:
                            spans = [(c0, 512), (512, C)]
                        else:
                            spans = [(c0, C)]
                        for a, b_ in spans:
                            nc.tensor.matmul(
                                st[:, a:b_],
                                kt_s[:, c0:c0 + P],
                                qt_s[:, a:b_],
                                start=True,
                                stop=True,
                            )
                        nc.scalar.activation(
                            pt_all[:, kt, c0:C],
                            st[:, c0:C],
                            mybir.ActivationFunctionType.Exp,
                            scale=SCALE,
                        )
                        # zero the strictly-upper (k > q) part of the diagonal block
                        nc.gpsimd.affine_select(
                            out=pt_all[:, kt, c0:c0 + P],
                            in_=pt_all[:, kt, c0:c0 + P],
                            compare_op=mybir.AluOpType.is_ge,
                            fill=0.0,
                            base=0,
                            channel_multiplier=-1,
                            pattern=[[1, P]],
                        )

                    # Phase 2: PV accumulation + normalize per query-tile
                    osb = opool.tile([P, T, D], F32, tag="osb")
                    for qt in range(T):
                        oacc = opsum.tile([P, D + 1], F32, tag="oacc")
                        for kt in range(qt + 1):
                            nc.tensor.matmul(
                                oacc,
                                pt_all[:, kt, qt * P:(qt + 1) * P],
                                vb[:, kt, :],
                                start=(kt == 0),
                                stop=(kt == qt),
                            )
                        den = small.tile([P, 1], F32, tag="den")
                        nc.vector.tensor_scalar_add(den, oacc[:, D:D + 1], es_t)
                        rec = small.tile([P, 1], F32, tag="rec")
                        nc.vector.reciprocal(rec, den)
                        nc.vector.tensor_scalar_mul(osb[:, qt, :], oacc[:, 0:D], rec)

                    nc.sync.dma_start(
                        out=out_d[pair, s0:s0 + C, :].rearrange(
                            "(t p) d -> p t d", p=P
                        ),
                        in_=osb,
                    )

    nc.compile()
    return nc


_PROGRAM = None


def _get_program():
    global _PROGRAM
    if _PROGRAM is None:
        _PROGRAM = _build_program()
    return _PROGRAM


def kernel(q, k, v, sinks, chunk_size):
    assert int(chunk_size) == C
    q = np.asarray(q, dtype=np.float32)
    k = np.asarray(k, dtype=np.float32)
    v = np.asarray(v, dtype=np.float32)
    sinks = np.asarray(sinks, dtype=np.float32)
    assert q.shape == (B, S, H, D)

    # [B,S,H,D] -> [B*H, S, D]
    qp = np.ascontiguousarray(q.transpose(0, 2, 1, 3)).reshape(PAIRS, S, D)
    kp = np.ascontiguousarray(k.transpose(0, 2, 1, 3)).reshape(PAIRS, S, D)
    vp = np.ascontiguousarray(v.transpose(0, 2, 1, 3)).reshape(PAIRS, S, D)
    es_pairs = np.tile(np.exp(sinks), B)  # es_pairs[i] = exp(sinks[i % H])
    esb = np.repeat(es_pairs[:, None], P, axis=1).astype(np.float32)

    in_maps = []
    for c in range(N_CORES):
        sl = slice(c * PPC, (c + 1) * PPC)
        in_maps.append(
            {"q": qp[sl], "k": kp[sl], "v": vp[sl], "esink": esb[sl]}
        )

    nc = _get_program()
    res = run_bass_kernel_spmd(nc, in_maps, core_ids=list(range(N_CORES)))

    outp = np.concatenate([res.results[c]["out"] for c in range(N_CORES)], axis=0)
    out = outp.reshape(B, H, S, D).transpose(0, 2, 1, 3)
    return np.ascontiguousarray(out)


# revision 9
# speedup vs baseline: 2.2400x; 2.2400x over previous
"""Chunked-causal attention with sinks on 8 TRN2 NeuronCores.

Sharding: the 64 (batch, head) pairs are split 8-per-core (data parallel on
B, tensor parallel on H). Each core runs the same Bass program over its 8
pairs x 4 chunks of 1024 tokens.

Per (pair, chunk) the kernel computes, entirely on-chip:
  Q^T, K^T  via TensorE transpose (fp32 in, the PSUM->SBUF copy casts bf16)
  S_T[k, q] = K @ Q^T          (TensorE, bf16, scores transposed so that the
                                PV matmul can consume exp(S_T) directly)
  P_T       = exp(S_T / sqrt(D))  (ScalarE; no max-subtraction -- softmax is
                                shift-invariant and scores here are O(5), so
                                fp32 exp is exact enough; masked entries are
                                zeroed after the exp)
  O[q, :]   = P_T^T @ [V | 1]  (TensorE; the ones column yields the softmax
                                denominator in column D of the same matmul)
  out       = O[:, :D] / (O[:, D] + exp(sink))
"""

import numpy as np

import concourse.bacc as bacc
import concourse.bass as bass
import concourse.mybir as mybir
import concourse.tile as tile
from concourse.bass_utils import run_bass_kernel_spmd
from concourse.masks import make_identity

N_CORES = 8
B, S, H, D = 4, 4096, 16, 128
C = 1024                # chunk size
NCH = S // C            # chunks per sequence
PAIRS = B * H           # 64 (batch, head) pairs
PPC = PAIRS // N_CORES  # pairs per core
P = 128                 # SBUF partitions
T = C // P              # 128-row tiles per chunk
SCALE = 1.0 / float(np.sqrt(D))

F32 = mybir.dt.float32
BF16 = mybir.dt.bfloat16


def _build_program(ppc=PPC, nch=NCH):
    s_len = nch * C
    nc = bacc.Bacc("TRN2", target_bir_lowering=False, debug=False)
    q_d = nc.dram_tensor("q", [ppc, s_len, D], F32, kind="ExternalInput")
    k_d = nc.dram_tensor("k", [ppc, s_len, D], F32, kind="ExternalInput")
    v_d = nc.dram_tensor("v", [ppc, s_len, D], F32, kind="ExternalInput")
    es_d = nc.dram_tensor("esink", [ppc, P], F32, kind="ExternalInput")
    out_d = nc.dram_tensor("out", [ppc, s_len, D], F32, kind="ExternalOutput")

    with tile.TileContext(nc) as tc:
        with (
            tc.tile_pool(name="consts", bufs=1) as consts,
            tc.tile_pool(name="loads", bufs=2) as loads,
            tc.tile_pool(name="vcast", bufs=2) as vpool,
            tc.tile_pool(name="qkt", bufs=2) as qktp,
            tc.tile_pool(name="ptile", bufs=2) as ppool,
            tc.tile_pool(name="outs", bufs=2) as opool,
            tc.tile_pool(name="small", bufs=4) as small,
            tc.tile_pool(name="spsum", bufs=2, space="PSUM") as spsum,
            tc.tile_pool(name="opsum", bufs=3, space="PSUM") as opsum,
            tc.tile_pool(name="tpsum", bufs=1, space="PSUM") as tpsum,
        ):
            ident = consts.tile([P, P], F32)
            make_identity(nc, ident)

            for pair in range(ppc):
                es_t = small.tile([P, 1], F32, tag="esink")
                nc.sync.dma_start(
                    out=es_t, in_=es_d[pair, :].rearrange("(p o) -> p o", o=1)
                )
                for ch in range(nch):
                    s0 = ch * C

                    qf = loads.tile([P, T, D], F32, tag="qf")
                    nc.sync.dma_start(
                        out=qf,
                        in_=q_d[pair, s0:s0 + C, :].rearrange("(t p) d -> p t d", p=P),
                    )
                    kf = loads.tile([P, T, D], F32, tag="kf")
                    nc.sync.dma_start(
                        out=kf,
                        in_=k_d[pair, s0:s0 + C, :].rearrange("(t p) d -> p t d", p=P),
                    )
                    vf = loads.tile([P, T, D], F32, tag="vf")
                    nc.sync.dma_start(
                        out=vf,
                        in_=v_d[pair, s0:s0 + C, :].rearrange("(t p) d -> p t d", p=P),
                    )

                    # V -> bf16 with a ones column (for the denominator)
                    vb = vpool.tile([P, T, D + 1], BF16, tag="vb")
                    nc.vector.tensor_copy(vb[:, :, 0:D], vf)
                    nc.gpsimd.memset(vb[:, :, D:D + 1], 1.0)

                    # Q^T / K^T: TensorE fp32 transposes; PSUM->SBUF copy
                    # casts to bf16. qkt[:, t, 0, :] = Q^T cols, [:, t, 1, :] = K^T.
                    qkt = qktp.tile([P, T, 2, P], BF16, tag="qkt")
                    for batch in range(T // 2):
                        tp = tpsum.tile([P, 2, 2, P], F32, tag="tp")
                        for tt in range(2):
                            t = batch * 2 + tt
                            nc.tensor.transpose(tp[:, tt, 0, :], qf[:, t, :], ident)
                            nc.tensor.transpose(tp[:, tt, 1, :], kf[:, t, :], ident)
                        nc.vector.tensor_copy(
                            qkt[:, batch * 2:batch * 2 + 2, :, :], tp
                        )

                    # Phase 1: scores + exp per key-tile
                    pt_all = ppool.tile([P, T, C], BF16, tag="pt")
                    for kt in range(T):
                        c0 = kt * P
                        st = spsum.tile([P, C], F32, tag="st")
                        # split matmuls at the PSUM bank boundary (col 512)
                        if c0 < 512:
                            spans = [(c0, 512), (512, C)]
                        else:
                            spans = [(c0, C)]
                        for a, b_ in spans:
                            nc.tensor.matmul(
                                st[:, a:b_],
                                qkt[:, kt, 1, :],
                                qkt[:, a // P:b_ // P, 0, :],
                                start=True,
                                stop=True,
                            )
                        nc.scalar.activation(
                            pt_all[:, kt, c0:C],
                            st[:, c0:C],
                            mybir.ActivationFunctionType.Exp,
                            scale=SCALE,
                        )
                        # zero the strictly-upper (k > q) part of the diagonal block
                        nc.gpsimd.affine_select(
                            out=pt_all[:, kt, c0:c0 + P],
                            in_=pt_all[:, kt, c0:c0 + P],
                            compare_op=mybir.AluOpType.is_ge,
                            fill=0.0,
                            base=0,
                            channel_multiplier=-1,
                            pattern=[[1, P]],
                        )

                    # Phase 2: PV accumulation + normalize, two query-tiles per
                    # PSUM bank (2 x 129 floats = 1032B fits one 2KB bank and
                    # neither matmul output crosses the bank boundary).
                    osb = opool.tile([P, T, D], F32, tag="osb")
                    for j in range(T // 2):
                        oacc = opsum.tile([P, 2, D + 1], F32, tag="oacc")
                        for qq in range(2):
                            qt = 2 * j + qq
                            for kt in range(qt + 1):
                                nc.tensor.matmul(
                                    oacc[:, qq, :],
                                    pt_all[:, kt, qt * P:(qt + 1) * P],
                                    vb[:, kt, :],
                                    start=(kt == 0),
                                    stop=(kt == qt),
                                )
                        den = small.tile([P, 2], F32, tag="den")
                        nc.vector.tensor_scalar_add(den, oacc[:, :, D], es_t)
                        rec = small.tile([P, 2], F32, tag="rec")
                        nc.vector.reciprocal(rec, den)
                        rec_b = bass.AP(
                            tensor=rec.tensor,
                            offset=rec.offset,
                            ap=[rec.ap[0], [1, 2], [0, D]],
                        )
                        nc.vector.tensor_tensor(
                            osb[:, 2 * j:2 * j + 2, :],
                            oacc[:, :, 0:D],
                            rec_b,
                            mybir.AluOpType.mult,
                        )

                    nc.sync.dma_start(
                        out=out_d[pair, s0:s0 + C, :].rearrange(
                            "(t p) d -> p t d", p=P
                        ),
                        in_=osb,
                    )

    nc.compile()
    return nc


_PROGRAM = None


def _get_program():
    global _PROGRAM
    if _PROGRAM is None:
        _PROGRAM = _build_program()
    return _PROGRAM


def kernel(q, k, v, sinks, chunk_size):
    assert int(chunk_size) == C
    q = np.asarray(q, dtype=np.float32)
    k = np.asarray(k, dtype=np.float32)
    v = np.asarray(v, dtype=np.float32)
    sinks = np.asarray(sinks, dtype=np.float32)
    assert q.shape == (B, S, H, D)

    # [B,S,H,D] -> [B*H, S, D]
    qp = np.ascontiguousarray(q.transpose(0, 2, 1, 3)).reshape(PAIRS, S, D)
    kp = np.ascontiguousarray(k.transpose(0, 2, 1, 3)).reshape(PAIRS, S, D)
    vp = np.ascontiguousarray(v.transpose(0, 2, 1, 3)).reshape(PAIRS, S, D)
    es_pairs = np.tile(np.exp(sinks), B)  # es_pairs[i] = exp(sinks[i % H])
    esb = np.repeat(es_pairs[:, None], P, axis=1).astype(np.float32)

    in_maps = []
    for c in range(N_CORES):
        sl = slice(c * PPC, (c + 1) * PPC)
        in_maps.append(
            {"q": qp[sl], "k": kp[sl], "v": vp[sl], "esink": esb[sl]}
        )

    nc = _get_program()
    res = run_bass_kernel_spmd(nc, in_maps, core_ids=list(range(N_CORES)))

    outp = np.concatenate([res.results[c]["out"] for c in range(N_CORES)], axis=0)
    out = outp.reshape(B, H, S, D).transpose(0, 2, 1, 3)
    return np.ascontiguousarray(out)


# revision 10
# speedup vs baseline: 2.4817x; 1.1079x over previous
"""Chunked-causal attention with sinks on 8 TRN2 NeuronCores.

Sharding: the 64 (batch, head) pairs are split 8-per-core (data parallel on
B, tensor parallel on H). Each core runs the same Bass program over its 8
pairs x 4 chunks of 1024 tokens.

The per-core shard layout is chosen for DMA/TensorE efficiency: Q and K are
handed to each core pre-transposed as bf16 [pairs, D, S] (the score matmul
contracts over D, which must sit on SBUF partitions; bf16 is the matmul
compute dtype either way -- the host conversion is numerically identical to
the on-device cast and makes the loads contiguous). V and the output stay
natural fp32 [pairs, S, D].

Per (pair, chunk) the kernel computes, entirely on-chip:
  S_T[k, q] = K @ Q^T          (TensorE, bf16; scores transposed so that the
                                PV matmul can consume exp(S_T) directly)
  P_T       = exp(S_T / sqrt(D))  (ScalarE; no max-subtraction -- softmax is
                                shift-invariant and scores here are O(5), so
                                fp32 exp is exact enough; masked entries are
                                zeroed after the exp)
  O[q, :]   = P_T^T @ [V | 1]  (TensorE; the ones column yields the softmax
                                denominator in column D of the same matmul)
  out       = O[:, :D] / (O[:, D] + exp(sink))
"""

import ml_dtypes
import numpy as np

import concourse.bacc as bacc
import concourse.bass as bass
import concourse.mybir as mybir
import concourse.tile as tile
from concourse.bass_utils import run_bass_kernel_spmd

N_CORES = 8
B, S, H, D = 4, 4096, 16, 128
C = 1024                # chunk size
NCH = S // C            # chunks per sequence
PAIRS = B * H           # 64 (batch, head) pairs
PPC = PAIRS // N_CORES  # pairs per core
P = 128                 # SBUF partitions
T = C // P              # 128-row tiles per chunk
SCALE = 1.0 / float(np.sqrt(D))

F32 = mybir.dt.float32
BF16 = mybir.dt.bfloat16


def _build_program(ppc=PPC, nch=NCH):
    s_len = nch * C
    nc = bacc.Bacc("TRN2", target_bir_lowering=False, debug=False)
    qt_d = nc.dram_tensor("qt", [ppc, D, s_len], BF16, kind="ExternalInput")
    kt_d = nc.dram_tensor("kt", [ppc, D, s_len], BF16, kind="ExternalInput")
    v_d = nc.dram_tensor("v", [ppc, s_len, D], F32, kind="ExternalInput")
    es_d = nc.dram_tensor("esink", [ppc, P], F32, kind="ExternalInput")
    out_d = nc.dram_tensor("out", [ppc, s_len, D], F32, kind="ExternalOutput")

    with tile.TileContext(nc) as tc:
        with (
            tc.tile_pool(name="loads", bufs=3) as loads,
            tc.tile_pool(name="vcast", bufs=2) as vpool,
            tc.tile_pool(name="ptile", bufs=2) as ppool,
            tc.tile_pool(name="outs", bufs=2) as opool,
            tc.tile_pool(name="small", bufs=4) as small,
            tc.tile_pool(name="spsum", bufs=2, space="PSUM") as spsum,
            tc.tile_pool(name="opsum", bufs=4, space="PSUM") as opsum,
        ):
            for pair in range(ppc):
                es_t = small.tile([P, 1], F32, tag="esink")
                nc.sync.dma_start(
                    out=es_t, in_=es_d[pair, :].rearrange("(p o) -> p o", o=1)
                )
                for ch in range(nch):
                    s0 = ch * C

                    qtb = loads.tile([P, C], BF16, tag="qtb")
                    nc.sync.dma_start(out=qtb, in_=qt_d[pair, :, s0:s0 + C])
                    ktb = loads.tile([P, C], BF16, tag="ktb")
                    nc.sync.dma_start(out=ktb, in_=kt_d[pair, :, s0:s0 + C])
                    vf = loads.tile([P, T, D], F32, tag="vf")
                    nc.sync.dma_start(
                        out=vf,
                        in_=v_d[pair, s0:s0 + C, :].rearrange("(t p) d -> p t d", p=P),
                    )

                    # V -> bf16 with a ones column (for the denominator)
                    vb = vpool.tile([P, T, D + 1], BF16, tag="vb")
                    nc.vector.tensor_copy(vb[:, :, 0:D], vf)
                    nc.gpsimd.memset(vb[:, :, D:D + 1], 1.0)

                    # Phase 1: scores + exp per key-tile
                    pt_all = ppool.tile([P, T, C], BF16, tag="pt")
                    for kt in range(T):
                        c0 = kt * P
                        st = spsum.tile([P, C], F32, tag="st")
                        # split matmuls at the PSUM bank boundary (col 512)
                        if c0 < 512:
                            spans = [(c0, 512), (512, C)]
                        else:
                            spans = [(c0, C)]
                        for a, b_ in spans:
                            nc.tensor.matmul(
                                st[:, a:b_],
                                ktb[:, c0:c0 + P],
                                qtb[:, a:b_],
                                start=True,
                                stop=True,
                            )
                        nc.scalar.activation(
                            pt_all[:, kt, c0:C],
                            st[:, c0:C],
                            mybir.ActivationFunctionType.Exp,
                            scale=SCALE,
                        )
                        # zero the strictly-upper (k > q) part of the diagonal block
                        nc.gpsimd.affine_select(
                            out=pt_all[:, kt, c0:c0 + P],
                            in_=pt_all[:, kt, c0:c0 + P],
                            compare_op=mybir.AluOpType.is_ge,
                            fill=0.0,
                            base=0,
                            channel_multiplier=-1,
                            pattern=[[1, P]],
                        )

                    # Phase 2: PV accumulation + normalize, two query-tiles per
                    # PSUM bank (2 x 129 floats = 1032B fits one 2KB bank and
                    # neither matmul output crosses the bank boundary).
                    osb = opool.tile([P, T, D], F32, tag="osb")
                    for j in range(T // 2):
                        oacc = opsum.tile([P, 2, D + 1], F32, tag="oacc")
                        for qq in range(2):
                            qt = 2 * j + qq
                            for kt in range(qt + 1):
                                nc.tensor.matmul(
                                    oacc[:, qq, :],
                                    pt_all[:, kt, qt * P:(qt + 1) * P],
                                    vb[:, kt, :],
                                    start=(kt == 0),
                                    stop=(kt == qt),
                                )
                        den = small.tile([P, 2], F32, tag="den")
                        nc.vector.tensor_scalar_add(den, oacc[:, :, D], es_t)
                        rec = small.tile([P, 2], F32, tag="rec")
                        nc.vector.reciprocal(rec, den)
                        rec_b = bass.AP(
                            tensor=rec.tensor,
                            offset=rec.offset,
                            ap=[rec.ap[0], [1, 2], [0, D]],
                        )
                        nc.vector.tensor_tensor(
                            osb[:, 2 * j:2 * j + 2, :],
                            oacc[:, :, 0:D],
                            rec_b,
                            mybir.AluOpType.mult,
                        )

                    nc.sync.dma_start(
                        out=out_d[pair, s0:s0 + C, :].rearrange(
                            "(t p) d -> p t d", p=P
                        ),
                        in_=osb,
                    )

    nc.compile()
    return nc


_PROGRAM = None


def _get_program():
    global _PROGRAM
    if _PROGRAM is None:
        _PROGRAM = _build_program()
    return _PROGRAM


def _prep_in_maps(q, k, v, sinks):
    # [B,S,H,D] -> [B*H, S, D]
    qp = np.ascontiguousarray(q.transpose(0, 2, 1, 3)).reshape(PAIRS, S, D)
    kp = np.ascontiguousarray(k.transpose(0, 2, 1, 3)).reshape(PAIRS, S, D)
    vp = np.ascontiguousarray(v.transpose(0, 2, 1, 3)).reshape(PAIRS, S, D)
    # Q, K additionally transposed to [pairs, D, S] bf16 (matmul layout/dtype)
    qT = np.ascontiguousarray(qp.transpose(0, 2, 1)).astype(ml_dtypes.bfloat16)
    kT = np.ascontiguousarray(kp.transpose(0, 2, 1)).astype(ml_dtypes.bfloat16)
    es_pairs = np.tile(np.exp(sinks), B)  # es_pairs[i] = exp(sinks[i % H])
    esb = np.repeat(es_pairs[:, None], P, axis=1).astype(np.float32)

    in_maps = []
    for c in range(N_CORES):
        sl = slice(c * PPC, (c + 1) * PPC)
        in_maps.append(
            {"qt": qT[sl], "kt": kT[sl], "v": vp[sl], "esink": esb[sl]}
        )
    return in_maps


def kernel(q, k, v, sinks, chunk_size):
    assert int(chunk_size) == C
    q = np.asarray(q, dtype=np.float32)
    k = np.asarray(k, dtype=np.float32)
    v = np.asarray(v, dtype=np.float32)
    sinks = np.asarray(sinks, dtype=np.float32)
    assert q.shape == (B, S, H, D)

    in_maps = _prep_in_maps(q, k, v, sinks)
    nc = _get_program()
    res = run_bass_kernel_spmd(nc, in_maps, core_ids=list(range(N_CORES)))

    outp = np.concatenate([res.results[c]["out"] for c in range(N_CORES)], axis=0)
    out = outp.reshape(B, H, S, D).transpose(0, 2, 1, 3)
    return np.ascontiguousarray(out)


# revision 15
# speedup vs baseline: 2.9492x; 1.1884x over previous
"""Chunked-causal attention with sinks on 8 TRN2 NeuronCores.

Sharding: the 64 (batch, head) pairs are split 8-per-core (data parallel on
B, tensor parallel on H). Each core runs the same Bass program over its 8
pairs x 4 chunks of 1024 tokens.

The per-core shard layout is chosen for DMA/TensorE efficiency: Q and K are
handed to each core pre-transposed as bf16 [pairs, D, S] (the score matmul
contracts over D, which must sit on SBUF partitions; bf16 is the matmul
compute dtype either way -- the host conversion is numerically identical to
the on-device cast and makes the loads contiguous). V and the output stay
natural fp32 [pairs, S, D].

Per (pair, chunk) the kernel computes, entirely on-chip:
  S_T[k, q] = K @ Q^T          (TensorE, bf16; scores transposed so that the
                                PV matmul can consume exp(S_T) directly)
  P_T       = exp(S_T / sqrt(D))  (ScalarE; no max-subtraction -- softmax is
                                shift-invariant and scores here are O(5), so
                                fp32 exp is exact enough; masked entries are
                                zeroed after the exp)
  O[q, :]   = P_T^T @ [V | 1]  (TensorE; the ones column yields the softmax
                                denominator in column D of the same matmul)
  out       = O[:, :D] / (O[:, D] + exp(sink))
"""

import ml_dtypes
import numpy as np

import concourse.bacc as bacc
import concourse.bass as bass
import concourse.mybir as mybir
import concourse.tile as tile
from concourse.bass_utils import run_bass_kernel_spmd

N_CORES = 8
B, S, H, D = 4, 4096, 16, 128
C = 1024                # chunk size
NCH = S // C            # chunks per sequence
PAIRS = B * H           # 64 (batch, head) pairs
PPC = PAIRS // N_CORES  # pairs per core
P = 128                 # SBUF partitions
T = C // P              # 128-row tiles per chunk
SCALE = 1.0 / float(np.sqrt(D))

F32 = mybir.dt.float32
BF16 = mybir.dt.bfloat16


def _build_program(ppc=PPC, nch=NCH):
    s_len = nch * C
    nc = bacc.Bacc("TRN2", target_bir_lowering=False, debug=False)
    qt_d = nc.dram_tensor("qt", [ppc, D, s_len], BF16, kind="ExternalInput")
    kt_d = nc.dram_tensor("kt", [ppc, D, s_len], BF16, kind="ExternalInput")
    # V and out use a partition-major [.., P, T, D] layout so each SBUF
    # partition's data is one contiguous 4KB run in DRAM (the host permutes).
    v_d = nc.dram_tensor("v", [ppc, nch, P, T, D], F32, kind="ExternalInput")
    es_d = nc.dram_tensor("esink", [ppc, P], F32, kind="ExternalInput")
    out_d = nc.dram_tensor("out", [ppc, nch, P, T, D], F32, kind="ExternalOutput")

    with tile.TileContext(nc) as tc:
        with (
            tc.tile_pool(name="loads", bufs=3) as loads,
            tc.tile_pool(name="vcast", bufs=2) as vpool,
            tc.tile_pool(name="ptile", bufs=2) as ppool,
            tc.tile_pool(name="outs", bufs=2) as opool,
            tc.tile_pool(name="small", bufs=4) as small,
            tc.tile_pool(name="spsum", bufs=2, space="PSUM") as spsum,
            tc.tile_pool(name="opsum", bufs=4, space="PSUM") as opsum,
        ):
            pending_store = [None]

            def flush_store():
                if pending_store[0] is not None:
                    osb_prev, pair_prev, ch_prev = pending_store[0]
                    nc.sync.dma_start(
                        out=out_d[pair_prev, ch_prev], in_=osb_prev
                    )
                    pending_store[0] = None

            for pair in range(ppc):
                es_t = small.tile([P, 1], F32, tag="esink")
                nc.sync.dma_start(
                    out=es_t, in_=es_d[pair, :].rearrange("(p o) -> p o", o=1)
                )
                for ch in range(nch):
                    s0 = ch * C

                    qtb = loads.tile([P, C], BF16, tag="qtb")
                    nc.sync.dma_start(out=qtb, in_=qt_d[pair, :, s0:s0 + C])
                    ktb = loads.tile([P, C], BF16, tag="ktb")
                    nc.sync.dma_start(out=ktb, in_=kt_d[pair, :, s0:s0 + C])
                    vf = loads.tile([P, T, D], F32, tag="vf")
                    nc.sync.dma_start(out=vf, in_=v_d[pair, ch])
                    flush_store()

                    # V -> bf16 with a ones column (for the denominator)
                    vb = vpool.tile([P, T, D + 1], BF16, tag="vb")
                    nc.vector.tensor_copy(vb[:, :, 0:D], vf)
                    nc.gpsimd.memset(vb[:, :, D:D + 1], 1.0)

                    # Phase 1: scores + exp per key-tile
                    pt_all = ppool.tile([P, T, C], BF16, tag="pt")
                    for kt in range(T):
                        c0 = kt * P
                        st = spsum.tile([P, C], F32, tag="st")
                        # split matmuls at the PSUM bank boundary (col 512)
                        if c0 < 512:
                            spans = [(c0, 512), (512, C)]
                        else:
                            spans = [(c0, C)]
                        for a, b_ in spans:
                            nc.tensor.matmul(
                                st[:, a:b_],
                                ktb[:, c0:c0 + P],
                                qtb[:, a:b_],
                                start=True,
                                stop=True,
                            )
                        nc.scalar.activation(
                            pt_all[:, kt, c0:C],
                            st[:, c0:C],
                            mybir.ActivationFunctionType.Exp,
                            scale=SCALE,
                        )
                        # zero the strictly-upper (k > q) part of the diagonal block
                        nc.gpsimd.affine_select(
                            out=pt_all[:, kt, c0:c0 + P],
                            in_=pt_all[:, kt, c0:c0 + P],
                            compare_op=mybir.AluOpType.is_ge,
                            fill=0.0,
                            base=0,
                            channel_multiplier=-1,
                            pattern=[[1, P]],
                        )

                    # Phase 2: PV accumulation + normalize, two query-tiles per
                    # PSUM bank (2 x 129 floats = 1032B fits one 2KB bank and
                    # neither matmul output crosses the bank boundary).
                    osb = opool.tile([P, T, D], F32, tag="osb")
                    for j in range(T // 2):
                        oacc = opsum.tile([P, 2, D + 1], F32, tag="oacc")
                        for qq in range(2):
                            qt = 2 * j + qq
                            for kt in range(qt + 1):
                                nc.tensor.matmul(
                                    oacc[:, qq, :],
                                    pt_all[:, kt, qt * P:(qt + 1) * P],
                                    vb[:, kt, :],
                                    start=(kt == 0),
                                    stop=(kt == qt),
                                )
                        den = small.tile([P, 2], F32, tag="den")
                        nc.vector.tensor_scalar_add(den, oacc[:, :, D], es_t)
                        rec = small.tile([P, 2], F32, tag="rec")
                        nc.vector.reciprocal(rec, den)
                        rec_b = bass.AP(
                            tensor=rec.tensor,
                            offset=rec.offset,
                            ap=[rec.ap[0], [1, 2], [0, D]],
                        )
                        nc.vector.tensor_tensor(
                            osb[:, 2 * j:2 * j + 2, :],
                            oacc[:, :, 0:D],
                            rec_b,
                            mybir.AluOpType.mult,
                        )

                    pending_store[0] = (osb, pair, ch)
            flush_store()

    nc.compile()
    return nc


_PROGRAM = None


def _get_program():
    global _PROGRAM
    if _PROGRAM is None:
        _PROGRAM = _build_program()
    return _PROGRAM


def _prep_in_maps(q, k, v, sinks):
    # [B,S,H,D] -> [B*H, S, D]
    qp = np.ascontiguousarray(q.transpose(0, 2, 1, 3)).reshape(PAIRS, S, D)
    kp = np.ascontiguousarray(k.transpose(0, 2, 1, 3)).reshape(PAIRS, S, D)
    vp = np.ascontiguousarray(v.transpose(0, 2, 1, 3)).reshape(PAIRS, S, D)
    # Q, K additionally transposed to [pairs, D, S] bf16 (matmul layout/dtype)
    qT = np.ascontiguousarray(qp.transpose(0, 2, 1)).astype(ml_dtypes.bfloat16)
    kT = np.ascontiguousarray(kp.transpose(0, 2, 1)).astype(ml_dtypes.bfloat16)
    # V permuted to [pairs, chunk, p, t, d] (s = t*P + p) for contiguous DMA
    vperm = np.ascontiguousarray(
        vp.reshape(PAIRS, NCH, T, P, D).transpose(0, 1, 3, 2, 4)
    )
    es_pairs = np.tile(np.exp(sinks), B)  # es_pairs[i] = exp(sinks[i % H])
    esb = np.repeat(es_pairs[:, None], P, axis=1).astype(np.float32)

    in_maps = []
    for c in range(N_CORES):
        sl = slice(c * PPC, (c + 1) * PPC)
        in_maps.append(
            {"qt": qT[sl], "kt": kT[sl], "v": vperm[sl], "esink": esb[sl]}
        )
    return in_maps


def kernel(q, k, v, sinks, chunk_size):
    assert int(chunk_size) == C
    q = np.asarray(q, dtype=np.float32)
    k = np.asarray(k, dtype=np.float32)
    v = np.asarray(v, dtype=np.float32)
    sinks = np.asarray(sinks, dtype=np.float32)
    assert q.shape == (B, S, H, D)

    in_maps = _prep_in_maps(q, k, v, sinks)
    nc = _get_program()
    res = run_bass_kernel_spmd(nc, in_maps, core_ids=list(range(N_CORES)))

    outp = np.concatenate([res.results[c]["out"] for c in range(N_CORES)], axis=0)
    # [pairs, chunk, p, t, d] -> [pairs, s, d] (s = chunk*C + t*P + p)
    outp = outp.transpose(0, 1, 3, 2, 4).reshape(PAIRS, S, D)
    out = outp.reshape(B, H, S, D).transpose(0, 2, 1, 3)
    return np.ascontiguousarray(out)
